# revision 39
# baseline (speedup 1.0000x reference)
"""Drosophila optic lobe circuit simulation on 8 Trainium2 NeuronCores.

Edge/target-sharded across 8 devices; batch rides partitions.
- N padded 49000->49152 = 8 dev x 8 k-groups x 768 targets.
- Partition p = 16k + 8x + b: k-group k, batch b, x in {0,1} a duplicate
  row pair (ap_gather shares one index stream per 16-partition core).
- r-table: [128, 49152] f16, the full relu(v) vector per batch, gathered
  as PAIRS (d=2, pair index src//2 fits int16); the wrong parity element
  is zeroed in the weights, so no source-half masking or fold matmul.
- currents = gathered-pairs * w2 (f16); scatter-add = fused pair-fold +
  carried inclusive cumsum (tensor_tensor_scan op0=add op1=add over the
  even/odd strided views) + boundary extraction (ap_gather per chunk) +
  first difference in extract-column space.
- Tables rebuilt ON DEVICE at the start of each step: gather v to
  id-order slab, relu+cast f16, DRAM, AllGather, broadcast DMAs.
- Host->device upload minimized (dominates wall time under axon): only
  per-edge indices/weights (deduped) + v0/mdt in small layouts.
"""

import numpy as np
import sys

sys.path.insert(0, "/opt/trn_rl_repo")

import concourse.bacc as bacc
import concourse.mybir as mybir
from concourse.tile import TileContext
from concourse.bass_utils import run_bass_kernel_spmd

NREAL = 49000
B = 8
DT = 0.1
NDEV = 8
N = 49152
NPAIR = N // 2
NDEVT = N // NDEV          # 6144
NCORES = 8
MCORE = NDEVT // NCORES    # 768
NCHUNK = 16
CHUNK = 1600
LCORE = NCHUNK * CHUNK     # 25600
WCH = 4                    # chunks per boundary window
BCH4 = 288                 # boundary capacity per 4-chunk (6400-slot) window
NWIN = NCHUNK // WCH       # 4
ECOLS = NWIN * BCH4        # 1152

_cache = {}


def _build(steps, with_bias, use_for_i=False):
    nc = bacc.Bacc(None)
    f32, f16, i16 = mybir.dt.float32, mybir.dt.float16, mybir.dt.int16

    idx1_in = nc.declare_dram_parameter("idx1", [128, LCORE // 16], i16, isOutput=False)
    w2_in = nc.declare_dram_parameter("w2", [8, 2 * LCORE], f16, isOutput=False)
    bidx_in = nc.declare_dram_parameter("bidx", [128, ECOLS // 16], i16, isOutput=False)
    idx3_in = nc.declare_dram_parameter("idx3", [128, MCORE // 16], i16, isOutput=False)
    v064_in = nc.declare_dram_parameter("v064", [64, ECOLS], f32, isOutput=False)
    mdt8_in = nc.declare_dram_parameter("mdt8", [8, ECOLS], f32, isOutput=False)
    if with_bias:
        bm8_in = nc.declare_dram_parameter("bm8", [8, ECOLS], f32, isOutput=False)
    # f16 output halves the round-trip (donated zero upload + download)
    vout = nc.declare_dram_parameter("vout", [B, NDEVT], f16, isOutput=True)

    with TileContext(nc) as tc:
        with (
            tc.tile_pool(name="big", bufs=1) as big,
            tc.tile_pool(name="gbuf", bufs=2) as gbuf,
            tc.tile_pool(name="wbuf", bufs=2) as wbuf,
            tc.tile_pool(name="cbuf", bufs=2) as cbuf,
            tc.tile_pool(name="small", bufs=1) as small,
            tc.tile_pool(name="dram", bufs=1, space="DRAM") as dram,
            tc.tile_pool(name="agpool", bufs=max(steps, 1), space="DRAM") as agp,
        ):
            tbl = big.tile([128, N], f16, tag="tbl")
            idx1 = small.tile([128, LCORE // 16], i16, tag="idx1")
            bidx = small.tile([128, ECOLS // 16], i16, tag="bidx")
            idx3 = small.tile([128, MCORE // 16], i16, tag="idx3")
            v = small.tile([128, ECOLS], f32, tag="v")
            mdt = small.tile([128, ECOLS], f32, tag="mdt")
            if with_bias:
                bm = small.tile([128, ECOLS], f32, tag="bm")
            else:
                bm = None
            # E has a permanent zero sentinel in column 0 so the first
            # difference needs no separate first-column copy
            E = small.tile([128, ECOLS + 1], f32, tag="E")
            D = small.tile([128, ECOLS], f32, tag="D")
            s4 = small.tile([128, WCH * CHUNK], f32, tag="s4")
            rslab = small.tile([128, MCORE], f32, tag="rslab")
            rs16 = small.tile([128, MCORE], f16, tag="rs16")
            vslab = small.tile([128, MCORE], f32, tag="vslab")
            vs16 = small.tile([128, MCORE], f16, tag="vs16")

            slab_d = dram.tile([B, NDEVT], f16)
            w2exp_d = dram.tile([128, 2 * LCORE], f16)

            nc.sync.dma_start(out=idx1[:], in_=idx1_in[:])
            nc.sync.dma_start(out=bidx[:], in_=bidx_in[:])
            nc.sync.dma_start(out=idx3[:], in_=idx3_in[:])
            nc.vector.memset(E[:, 0:1], 0.0)
            # expand w2 [8, 2*LCORE] (row k) to the 16 partitions of core k
            # (DRAM->DRAM broadcast-8, twice per core)
            for k in range(8):
                for x in range(2):
                    p0 = 16 * k + 8 * x
                    nc.sync.dma_start(
                        out=w2exp_d[p0:p0 + 8, :],
                        in_=w2_in[:][k, :][None].to_broadcast([8, 2 * LCORE]),
                    )
                    # expand v064 (rows 8k+b) / mdt8 / bm8 into SBUF
                    nc.sync.dma_start(out=v[p0:p0 + 8, :],
                                      in_=v064_in[8 * k:8 * k + 8, :])
                    nc.sync.dma_start(
                        out=mdt[p0:p0 + 8, :],
                        in_=mdt8_in[:][k, :][None].to_broadcast([8, ECOLS]),
                    )
                    if with_bias:
                        nc.sync.dma_start(
                            out=bm[p0:p0 + 8, :],
                            in_=bm8_in[:][k, :][None].to_broadcast([8, ECOLS]),
                        )

            def step_body():
                # --- rebuild r-tables from current v ---
                nc.gpsimd.ap_gather(out_ap=rslab[:], in_ap=v[:], idxs_ap=idx3[:],
                                    channels=128, num_elems=ECOLS, d=1, num_idxs=MCORE)
                nc.vector.tensor_scalar(out=rs16[:], in0=rslab[:], scalar1=0.0,
                                        scalar2=None, op0=mybir.AluOpType.max)
                for k in range(NCORES):
                    nc.sync.dma_start(out=slab_d[:, k * MCORE:(k + 1) * MCORE],
                                      in_=rs16[16 * k:16 * k + 8, :])
                ag_d = agp.tile([NDEV * B, NDEVT], f16, addr_space="Shared", tag="ag")
                nc.gpsimd.collective_compute(
                    "AllGather", mybir.AluOpType.bypass,
                    replica_groups=[list(range(NDEV))],
                    ins=[slab_d[:]], outs=[ag_d[:]],
                )
                agv = ag_d[:].rearrange("(d b) n -> d b n", d=NDEV)
                for x in range(2):
                    for b in range(B):
                        nc.sync.dma_start(
                            out=tbl[:].rearrange("(k r) n -> k r n", k=8)[:, 8 * x + b, :],
                            in_=agv[:, b, :][None].to_broadcast([8, NDEV, NDEVT]),
                        )

                # --- edge chunks: gather pairs, mask-mult, pair-fold + scan ---
                # Four chunks share one persistent scan tile so boundary
                # extraction needs only 4 POOL gathers per step.
                for w in range(NWIN):
                    for half in range(WCH):
                        c = WCH * w + half
                        g2 = gbuf.tile([128, 2 * CHUNK], f16, tag="g2")
                        w2t = wbuf.tile([128, 2 * CHUNK], f16, tag="w2t")
                        cur2 = cbuf.tile([128, 2 * CHUNK], f16, tag="cur2")
                        nc.sync.dma_start(
                            out=w2t[:],
                            in_=w2exp_d[:, c * 2 * CHUNK:(c + 1) * 2 * CHUNK])
                        nc.gpsimd.ap_gather(
                            out_ap=g2[:], in_ap=tbl[:],
                            idxs_ap=idx1[:, c * (CHUNK // 16):(c + 1) * (CHUNK // 16)],
                            channels=128, num_elems=NPAIR, d=2, num_idxs=CHUNK,
                        )
                        if half == 0 and w >= 1:
                            nc.gpsimd.ap_gather(
                                out_ap=E[:, 1 + (w - 1) * BCH4:1 + w * BCH4],
                                in_ap=s4[:],
                                idxs_ap=bidx[:, (w - 1) * (BCH4 // 16):w * (BCH4 // 16)],
                                channels=128, num_elems=WCH * CHUNK, d=1, num_idxs=BCH4,
                            )
                        nc.vector.tensor_tensor(out=cur2[:], in0=g2[:], in1=w2t[:],
                                                op=mybir.AluOpType.mult)
                        c2v = cur2[:].rearrange("p (t q) -> p t q", q=2)
                        if half == 0:
                            init = 0.0 if w == 0 else \
                                s4[:, WCH * CHUNK - 1:WCH * CHUNK]
                        else:
                            init = s4[:, half * CHUNK - 1:half * CHUNK]
                        nc.vector.tensor_tensor_scan(
                            out=s4[:, half * CHUNK:(half + 1) * CHUNK],
                            data0=c2v[:, :, 0], data1=c2v[:, :, 1],
                            initial=init,
                            op0=mybir.AluOpType.add, op1=mybir.AluOpType.add,
                        )
                w = NWIN
                nc.gpsimd.ap_gather(
                    out_ap=E[:, 1 + (w - 1) * BCH4:1 + w * BCH4], in_ap=s4[:],
                    idxs_ap=bidx[:, (w - 1) * (BCH4 // 16):w * (BCH4 // 16)],
                    channels=128, num_elems=WCH * CHUNK, d=1, num_idxs=BCH4,
                )
                # --- segment sums by first difference; v update ---
                nc.vector.tensor_tensor(out=D[:], in0=E[:, 1:ECOLS + 1],
                                        in1=E[:, 0:ECOLS],
                                        op=mybir.AluOpType.subtract)
                nc.vector.tensor_tensor(out=D[:], in0=D[:], in1=v[:],
                                        op=mybir.AluOpType.subtract)
                nc.vector.tensor_tensor(out=D[:], in0=D[:], in1=mdt[:],
                                        op=mybir.AluOpType.mult)
                nc.vector.tensor_tensor(out=v[:], in0=v[:], in1=D[:],
                                        op=mybir.AluOpType.add)
                if with_bias:
                    nc.vector.tensor_tensor(out=v[:], in0=v[:], in1=bm[:],
                                            op=mybir.AluOpType.add)

            if use_for_i and steps > 1:
                with tc.For_i(0, steps, 1):
                    step_body()
            else:
                for _ in range(steps):
                    step_body()

            nc.gpsimd.ap_gather(out_ap=vslab[:], in_ap=v[:], idxs_ap=idx3[:],
                                channels=128, num_elems=ECOLS, d=1, num_idxs=MCORE)
            nc.vector.tensor_copy(out=vs16[:], in_=vslab[:])
            for k in range(NCORES):
                nc.sync.dma_start(out=vout[:, k * MCORE:(k + 1) * MCORE],
                                  in_=vs16[16 * k:16 * k + 8, :])
    nc.finalize()
    return nc


def _wrap16(a):
    out = np.zeros((128, a.shape[1] // 16), a.dtype)
    for k in range(8):
        for p in range(16):
            out[16 * k + p] = a[k, p::16]
    return out


def _prep(tm1_input, v_init, weights, bias, tau_params, scale_excitatory,
          scale_inhibitory, source_indices, target_indices, type_ids,
          tm1_indices, steps):
    one = np.float32(1.0)
    weights = np.asarray(weights, np.float32)
    es = np.where(weights > 0, np.float32(scale_excitatory),
                  np.where(weights < 0, np.float32(scale_inhibitory), one))
    sw = (weights * es).astype(np.float32)

    type_ids = np.asarray(type_ids)
    tau = np.asarray(tau_params, np.float32)[type_ids]
    taup = np.concatenate([tau, np.full(N - NREAL, 1.0, np.float32)])
    is_tm1 = np.zeros(N, bool)
    tm1_indices = np.asarray(tm1_indices)
    is_tm1[tm1_indices] = True
    biasp = np.zeros(N, np.float32)
    biasp[:NREAL] = np.asarray(bias, np.float32)

    vc = np.zeros((B, N), np.float32)
    vc[:, :NREAL] = np.asarray(v_init, np.float32)
    vc[:, tm1_indices] = np.asarray(tm1_input, np.float32)

    order = np.argsort(target_indices, kind="stable")
    tsrc = np.asarray(source_indices)[order].astype(np.int64)
    tw = sw[order]
    ttgt = np.asarray(target_indices)[order].astype(np.int64)
    t_starts = np.searchsorted(ttgt, np.arange(N + 1, dtype=np.int64), side="left")

    in_maps = []
    meta = []
    for d in range(NDEV):
        idx1 = np.zeros((8, LCORE), np.int16)
        w2 = np.zeros((8, 2 * LCORE), np.float16)
        bpos = np.zeros((8, ECOLS), np.int16)
        col_of_t = np.zeros((8, MCORE), np.int64)
        for k in range(NCORES):
            t0 = d * NDEVT + k * MCORE
            e0, e1 = t_starts[t0], t_starts[t0 + MCORE]
            srcs = tsrc[e0:e1]
            ws = tw[e0:e1]
            counts = t_starts[t0 + 1:t0 + MCORE + 1] - t_starts[t0:t0 + MCORE]
            pos = np.cumsum(counts)              # extract position per target
            nslots = 1 + len(srcs)               # sentinel at slot 0
            assert nslots <= LCORE, f"core slots {nslots} > {LCORE}"
            slots = 1 + np.arange(len(srcs))
            idx1[k, 1:nslots] = (srcs // 2).astype(np.int16)
            w2[k, 2 * slots + (srcs % 2)] = ws.astype(np.float16)
            # boundary extraction, chunked (real targets only; virtual
            # padding targets share the final pad column: syn there is
            # garbage but mdt=0 and v0=0 keep their state at 0)
            ids_k = d * NDEVT + k * MCORE + np.arange(MCORE)
            cchunk = pos // (WCH * CHUNK)
            clocal = pos % (WCH * CHUNK)
            ci = 0
            for c in range(NWIN):
                nhere = 0
                while ci < MCORE and cchunk[ci] == c:
                    if ids_k[ci] >= NREAL:
                        col_of_t[k, ci] = ECOLS - 1
                        ci += 1
                        continue
                    assert nhere < BCH4 - 1, f"window {c} boundary overflow"
                    bpos[k, c * BCH4 + nhere] = clocal[ci]
                    col_of_t[k, ci] = c * BCH4 + nhere
                    nhere += 1
                    ci += 1
                padv = bpos[k, c * BCH4 + nhere - 1] if nhere else 0
                bpos[k, c * BCH4 + nhere:(c + 1) * BCH4] = padv
            assert ci == MCORE
        gids = (d * NDEVT + np.arange(NDEVT)).reshape(NCORES, MCORE)
        v064 = np.zeros((64, ECOLS), np.float32)
        mdt8 = np.zeros((8, ECOLS), np.float32)
        bm8 = np.zeros((8, ECOLS), np.float32)
        for k in range(NCORES):
            cols = col_of_t[k]
            ids = gids[k]
            upd = (~is_tm1[ids]) & (ids < NREAL)
            mvals = np.where(upd, DT / taup[ids], 0.0).astype(np.float32)
            mdt8[k, cols] = mvals
            bm8[k, cols] = (mvals * biasp[ids]).astype(np.float32)
            for b in range(B):
                v064[8 * k + b, cols] = vc[b, ids]
        in_maps.append({
            "idx1": _wrap16(idx1), "w2": w2,
            "bidx": _wrap16(bpos), "idx3": _wrap16(col_of_t.astype(np.int16)),
            "v064": v064, "mdt8": mdt8, "bm8": bm8,
        })
        meta.append(col_of_t)
    return in_maps, meta


def kernel(**inputs):
    steps = int(inputs["steps"])
    bias = np.asarray(inputs["bias"])
    with_bias = bool(np.any(bias != 0))
    in_maps, _meta = _prep(**inputs)
    if not with_bias:
        for m in in_maps:
            m.pop("bm8")
    key = (steps, with_bias)
    if key not in _cache:
        _cache[key] = _build(steps, with_bias)
    nc = _cache[key]
    res = run_bass_kernel_spmd(nc, in_maps, list(range(NDEV)))
    out = np.zeros((B, NREAL), np.float32)
    for d in range(NDEV):
        sl = res.results[d]["vout"]
        lo = d * NDEVT
        hi = min(lo + NDEVT, NREAL)
        out[:, lo:hi] = sl[:, :hi - lo].astype(np.float32)
    return out
